# revision 1
# baseline (speedup 1.0000x reference)
"""Trainium2 Bass kernel for depthwise morphological dilation 2D (max-plus).

Problem (hardcoded):
  x:      (4, 256, 256, 64)  f32
  kernel: (3, 3, 64, 2)      f32
  out[b,y,x, di*64+c] = max_{i,j} x[b, y+i-1, x+j-1, c] + kernel[i,j,c,di]
  (SAME padding, stride 1), out: (4, 256, 256, 128) f32

Sharding: 8 cores, core k -> (batch b = k//2, H-half g2 = k%2), each core
computes 128 output rows (halo of 1 row each side handled host-side by
overlapped slicing of a padded array).

Device layout: SBUF partition p = g*64 + c  (g in {0,1} = row-subgroup of 64
output rows, c = channel). Free dims = (h, w) plane of one channel. The
per-channel kernel taps become per-partition [128,1] scalars.

Per tap (i,j,di): tmp = x_plane[i:i+R, j:j+256] + k[i,j,c,di]; acc = max(acc, tmp).
Adds run on ScalarE (activation Identity with per-partition bias; j==1 taps
must go there since their odd-element offset breaks DVE's 4B-alignment fast
modes) and on DVE (tensor_scalar at 4x). Maxes run on DVE (tensor_tensor at
2x, 16-bit). This 6-ACT/3-DVE tap split balances both engines at ~165us —
the stock-op floor (8 merges/di are DVE-only at 0.5 cyc/elem; fused
add+max alternatives all run at 1 elem/cyc and lose).

Schedule refinements over the original baseline:
  - ktab DMA issued on the ACT HWDGE queue so the block0 x DMA heads the
    SP queue (shaves the compute-start ramp).
  - last block-di uses a 5-ACT/4-DVE split (ACT retires earlier) and its
    final max + output DMA are split into two row-halves so the first
    half's DMA overlaps the second half's max (shaves the tail).
Measured (sim single-shot): 182.7us vs baseline 190.6us; correctness on
HW: rel_err 3.4e-04.

Compute dtype fp16 (max-plus accumulates no rounding error — only
input/one-sum rounding, ~3e-4 rel err; fp16 pad is -60000, within range).
Outputs stored 16-bit and widened to f32 on host.
"""

import os
import sys

import numpy as np

# The Bass kernel executes via the axon/neuron PJRT backend; a CPU pin (used
# for running jax references) would hide the NeuronCores. Only effective if
# jax has not been imported yet in this process.
if "jax" not in sys.modules and os.environ.get("JAX_PLATFORMS") == "cpu":
    os.environ["JAX_PLATFORMS"] = ""

for _p in ("/opt/trn_rl_repo",):
    if _p not in sys.path:
        sys.path.insert(0, _p)

import ml_dtypes

from concourse import mybir
import concourse.bass as bass
import concourse.tile as tile
from concourse.bass_utils import run_bass_kernel_spmd

BF16 = ml_dtypes.bfloat16

B, H, W, C = 4, 256, 256, 64
KH, KW, DM = 3, 3, 2
N_CORES = 8
# -1e30 for f32/bf16; fp16 uses -60000 (finite, far below any real x+k)
PAD_VAL = np.float32(-1e30)
PAD_VAL_F16 = np.float32(-60000.0)

# device-side tiling
G = 2          # partition row-subgroups
HG = 64        # output rows per subgroup (per core: G*HG = 128)
WP = W + 2     # padded width
XROWS = HG + 2  # input rows per subgroup

KERNEL_DT = "f16"

BLOCKS = [(0, 32), (32, 32)]

# per-(block,di) engine split of the 8 non-init taps: (ACT taps, DVE taps).
INIT_TAP = (1, 0)
A6 = ([(0, 1), (1, 1), (2, 1), (0, 0), (2, 2), (0, 2)], [(2, 0), (1, 2)])
A5 = ([(0, 1), (1, 1), (2, 1), (0, 0), (2, 2)], [(2, 0), (1, 2), (0, 2)])

_CACHED = {}


def _legalize_waits(nc):
    """Split multi-wait instructions: the TPB ISA allows one sem-wait per
    regular instruction (EventSemaphore holds 2). Tile's scheduler attaches
    all required waits to the consuming instruction; hoist the extras onto
    EventSemaphore instructions inserted just before it on the same engine
    queue (the queue is in-order, so semantics are unchanged)."""
    n_extra = 0
    for fn in nc.m.functions:
        for blk in fn.blocks:
            insts = blk.instructions
            new_list = []
            for ins in insts:
                si = ins.sync_info
                waits = list(si.on_wait) if (si and si.on_wait) else []
                if len(waits) > 1 and ins.opcode != "EventSemaphore":
                    keep, extra = waits[-1], waits[:-1]
                    for k in range(0, len(extra), 2):
                        es = mybir.InstEventSemaphore(
                            name=f"legalw_{ins.name}_{k}", ins=[], outs=[]
                        )
                        es.engine = ins.engine
                        es.sync_info = mybir.SyncInfo(
                            on_wait=extra[k : k + 2], on_update=[]
                        )
                        new_list.append(es)
                        n_extra += 1
                    si.on_wait = [keep]
                new_list.append(ins)
            insts[:] = new_list
    return n_extra


def _build_bass(dt_name: str, legalize: bool = True):
    repeat = int(os.environ.get("BASS_KERNEL_REPEAT", "1"))
    key = (dt_name, legalize, repeat)
    if key in _CACHED:
        return _CACHED[key]
    dt = {
        "bf16": mybir.dt.bfloat16,
        "f16": mybir.dt.float16,
        "f32": mybir.dt.float32,
    }[dt_name]

    nc = bass.Bass(
        "TRN2",
        target_bir_lowering=False,
        debug=False,
        num_devices=N_CORES,
    )
    x_ap = nc.dram_tensor("x", [128, XROWS, WP], dt, kind="ExternalInput").ap()
    kt_ap = nc.dram_tensor(
        "ktab", [128, KH * KW * DM], mybir.dt.float32, kind="ExternalInput"
    ).ap()
    o_aps = [
        nc.dram_tensor(f"o{di}", [128, HG, W], dt, kind="ExternalOutput").ap()
        for di in range(DM)
    ]

    from contextlib import ExitStack

    with tile.TileContext(nc) as tc, ExitStack() as ctx:
        kpool = ctx.enter_context(tc.tile_pool(name="kp", bufs=1))
        xpool = ctx.enter_context(tc.tile_pool(name="xp", bufs=2))
        tpool = ctx.enter_context(tc.tile_pool(name="tp", bufs=6))
        apool = ctx.enter_context(tc.tile_pool(name="acp", bufs=3))

        for _rep in range(repeat):
            # ktab rides the ACT HWDGE queue (tiny + every compute op reads
            # it) so block0's x DMA heads the SP queue.
            ktile = kpool.tile(
                [128, KH * KW * DM], mybir.dt.float32, name="ktile"
            )
            nc.scalar.dma_start(ktile[:], kt_ap[:])

            xbs = []
            for r0, rows in BLOCKS:
                xb = xpool.tile([128, rows + 2, WP], dt, name="xb", tag="xb")
                nc.sync.dma_start(xb[:], x_ap[:, r0 : r0 + rows + 2, :])
                xbs.append(xb)

            # ACT warmup: zero-dependency ACTIVATE to hang the
            # activation-table load on, overlapping the input DMA.
            warm = kpool.tile([128, 1], dt, name="warm")
            nc.gpsimd.memset(warm[:], 0.0)
            nc.scalar.add(warm[:], warm[:], 0.0)

            def kvec(i, j, di):
                t = di * 9 + i * 3 + j
                return ktile[:, t : t + 1]

            n_bd = len(BLOCKS) * DM
            bd = 0
            for blk, (r0, rows) in enumerate(BLOCKS):
                xb = xbs[blk]

                def src(i, j):
                    return xb[:, i : i + rows, j : j + W]

                for di in range(DM):
                    bd += 1
                    last_bd = bd == n_bd
                    act_taps, dve_taps = A5 if last_bd else A6
                    acc = apool.tile([128, rows, W], dt, name="acc", tag="acc")
                    i, j = INIT_TAP
                    nc.vector.tensor_scalar_add(
                        acc[:], src(i, j), kvec(i, j, di)
                    )
                    tmps = []
                    for i, j in dve_taps:
                        tmpv = tpool.tile(
                            [128, rows, W], dt, name="tmpv", tag="tmp"
                        )
                        nc.vector.tensor_scalar_add(
                            tmpv[:], src(i, j), kvec(i, j, di)
                        )
                        tmps.append(tmpv)
                    for i, j in act_taps:
                        tmpa = tpool.tile(
                            [128, rows, W], dt, name="tmpa", tag="tmp"
                        )
                        nc.scalar.add(tmpa[:], src(i, j), kvec(i, j, di))
                        tmps.append(tmpa)
                    if not last_bd:
                        for t in tmps:
                            nc.vector.tensor_max(acc[:], acc[:], t[:])
                        nc.sync.dma_start(
                            o_aps[di][:, r0 : r0 + rows, :], acc[:]
                        )
                    else:
                        # tail: split the final max + DMA into two
                        # row-halves so the first half's output DMA
                        # overlaps the second half's max.
                        for t in tmps[:-1]:
                            nc.vector.tensor_max(acc[:], acc[:], t[:])
                        t = tmps[-1]
                        h0 = rows // 2
                        for s0, s1 in ((0, h0), (h0, rows)):
                            nc.vector.tensor_max(
                                acc[:, s0:s1], acc[:, s0:s1], t[:, s0:s1]
                            )
                            nc.sync.dma_start(
                                o_aps[di][:, r0 + s0 : r0 + s1, :],
                                acc[:, s0:s1],
                            )

    if legalize:
        _legalize_waits(nc)
    _CACHED[key] = nc
    return nc


def _host_prep(x: np.ndarray, kern: np.ndarray, dt_name: str):
    """Build per-core input maps."""
    np_dt = {"bf16": BF16, "f16": np.float16, "f32": np.float32}[dt_name]
    pad = PAD_VAL_F16 if dt_name == "f16" else PAD_VAL
    xp = np.full((B, H + 2, W + 2, C), pad, np.float32)
    xp[:, 1 : H + 1, 1 : W + 1, :] = x
    # (B, C, H+2, W+2), cast first to halve the transpose copy
    xp_t = np.ascontiguousarray(xp.astype(np_dt).transpose(0, 3, 1, 2))

    # ktab[g*64+c, di*9+i*3+j] = kern[i,j,c,di]
    ktap = np.transpose(kern.astype(np.float32), (2, 3, 0, 1)).reshape(C, DM * 9)
    ktab = np.ascontiguousarray(np.tile(ktap, (G, 1)))

    in_maps = []
    for core in range(N_CORES):
        b, g2 = core // 2, core % 2
        xa = np.empty((128, XROWS, WP), np_dt)
        for g in range(G):
            r0 = g2 * 128 + g * HG
            xa[g * C : (g + 1) * C] = xp_t[b, :, r0 : r0 + XROWS, :]
        in_maps.append({"x": xa, "ktab": ktab})
    return in_maps


def _assemble(results):
    out = np.empty((B, H, W, DM * C), np.float32)
    for core in range(N_CORES):
        b, g2 = core // 2, core % 2
        for di in range(DM):
            o = np.asarray(results[core][f"o{di}"]).astype(np.float32)
            o4 = o.reshape(G, C, HG, W)
            for g in range(G):
                h0 = g2 * 128 + g * HG
                out[b, h0 : h0 + HG, :, di * C : (di + 1) * C] = o4[g].transpose(
                    1, 2, 0
                )
    return out


def run(x, kern, trace=False):
    """Run on hardware; returns (output, BassKernelResults)."""
    nc = _build_bass(KERNEL_DT)
    in_maps = _host_prep(np.asarray(x, np.float32), np.asarray(kern, np.float32),
                         KERNEL_DT)
    r = run_bass_kernel_spmd(nc, in_maps, list(range(N_CORES)), trace=trace)
    return _assemble(r.results), r


def kernel(x, kernel):
    out, _ = run(x, kernel)
    return out



# revision 21
# speedup vs baseline: 1.0196x; 1.0196x over previous
"""Trainium2 Bass kernel for depthwise morphological dilation 2D (max-plus).

Problem (hardcoded):
  x:      (4, 256, 256, 64)  f32
  kernel: (3, 3, 64, 2)      f32
  out[b,y,x, di*64+c] = max_{i,j} x[b, y+i-1, x+j-1, c] + kernel[i,j,c,di]
  (SAME padding, stride 1), out: (4, 256, 256, 128) f32

Sharding: 8 cores, core k -> (batch b = k//2, H-half g2 = k%2), each core
computes 128 output rows (halo of 1 row each side handled host-side by
overlapped slicing of a padded array).

Device layout: SBUF partition p = g*64 + c  (g in {0,1} = row-subgroup of 64
output rows, c = channel). Free dims = (h, w) plane of one channel. The
per-channel kernel taps become per-partition [128,1] scalars.

Per tap (i,j,di): tmp = x_plane[i:i+R, j:j+256] + k[i,j,c,di]; acc = max(acc, tmp).
Adds run on ScalarE (activation Identity with per-partition bias; j==1 taps
must go there since their odd-element offset breaks DVE's 4B-alignment fast
modes) and on DVE (tensor_scalar at 4x). Maxes run on DVE (tensor_tensor at
2x, 16-bit). This 6-ACT/3-DVE tap split balances both engines at ~165us —
the stock-op floor (8 merges/di are DVE-only at 0.5 cyc/elem; fused
add+max alternatives all run at 1 elem/cyc and lose).

Schedule refinements over the original baseline:
  - ktab DMA issued on the ACT HWDGE queue so the block0 x DMA heads the
    SP queue (shaves the compute-start ramp).
  - last block-di uses a 5-ACT/4-DVE split (ACT retires earlier) and its
    final max + output DMA are split into two row-halves so the first
    half's DMA overlaps the second half's max (shaves the tail).
Measured (sim single-shot): 182.7us vs baseline 190.6us; correctness on
HW: rel_err 3.4e-04.

Compute dtype fp16 (max-plus accumulates no rounding error — only
input/one-sum rounding, ~3e-4 rel err; fp16 pad is -60000, within range).
Outputs stored 16-bit and widened to f32 on host.
"""

import os
import sys

import numpy as np

# The Bass kernel executes via the axon/neuron PJRT backend; a CPU pin (used
# for running jax references) would hide the NeuronCores. Only effective if
# jax has not been imported yet in this process.
if "jax" not in sys.modules and os.environ.get("JAX_PLATFORMS") == "cpu":
    os.environ["JAX_PLATFORMS"] = ""

for _p in ("/opt/trn_rl_repo",):
    if _p not in sys.path:
        sys.path.insert(0, _p)

import ml_dtypes

from concourse import mybir
import concourse.bass as bass
import concourse.tile as tile
from concourse.bass_utils import run_bass_kernel_spmd

BF16 = ml_dtypes.bfloat16

B, H, W, C = 4, 256, 256, 64
KH, KW, DM = 3, 3, 2
N_CORES = 8
# -1e30 for f32/bf16; fp16 uses -60000 (finite, far below any real x+k)
PAD_VAL = np.float32(-1e30)
PAD_VAL_F16 = np.float32(-60000.0)

# device-side tiling
G = 2          # partition row-subgroups
HG = 64        # output rows per subgroup (per core: G*HG = 128)
WP = W + 2     # padded width
XROWS = HG + 2  # input rows per subgroup

KERNEL_DT = "f16"

# 16-row blocks: small enough that two units' worth of tmp tiles fit in
# SBUF (needed for the software-pipelined emission below), large enough
# to amortize ACT's 222-cycle per-op access overhead.
RB = 16
BLOCKS = [(r0, RB) for r0 in range(0, HG, RB)]
TAIL_SPLIT = 8  # last unit's merges+DMA run in 8-row slices

# Tap → engine assignment, per (block, di) unit.
#
# Work per unit: 9 tap adds (+k, per-partition scalar) and an 8-way max
# merge. Only DVE and ACT can do this work on the walrus/BIR compile
# path (Pool/gpsimd elementwise ops fail the v3 ISA engine check; DMA
# CCE rejects max; custom DVE ops are modeled/uncoded at 1x):
#   ACT — adds at 1 elem/cyc @1.2GHz; takes the j==1 taps (odd 2B
#         offset breaks DVE's 4B-alignment fast modes).
#   DVE — tensor_scalar adds at 4x, tensor_tensor maxes at 2x (fp16).
# LP balance: ~46 of the 72 adds on ACT, the rest + all 64 maxes on
# DVE → both engines ~167us busy.
INIT_TAP = (1, 0)
J1_TAPS = [(0, 1), (1, 1), (2, 1)]
ALIGNED = [(0, 0), (0, 2), (2, 0), (2, 2), (1, 2)]  # minus INIT_TAP
# ACT adds per unit (blk-major, di-minor); total 47 (ACT carries a
# little extra — it has tail slack while DVE is the binding engine).
# The last unit runs lighter ACT so its queue drains before the tail.
UNIT_PLAN = [6, 6, 6, 6, 6, 6, 5, 5]

_CACHED = {}


def _legalize_waits(nc):
    """Split multi-wait instructions: the TPB ISA allows one sem-wait per
    regular instruction (EventSemaphore holds 2). Tile's scheduler attaches
    all required waits to the consuming instruction; hoist the extras onto
    EventSemaphore instructions inserted just before it on the same engine
    queue (the queue is in-order, so semantics are unchanged)."""
    n_extra = 0
    for fn in nc.m.functions:
        for blk in fn.blocks:
            insts = blk.instructions
            new_list = []
            for ins in insts:
                si = ins.sync_info
                waits = list(si.on_wait) if (si and si.on_wait) else []
                if len(waits) > 1 and ins.opcode != "EventSemaphore":
                    keep, extra = waits[-1], waits[:-1]
                    for k in range(0, len(extra), 2):
                        es = mybir.InstEventSemaphore(
                            name=f"legalw_{ins.name}_{k}", ins=[], outs=[]
                        )
                        es.engine = ins.engine
                        es.sync_info = mybir.SyncInfo(
                            on_wait=extra[k : k + 2], on_update=[]
                        )
                        new_list.append(es)
                        n_extra += 1
                    si.on_wait = [keep]
                new_list.append(ins)
            insts[:] = new_list
    return n_extra


def _build_bass(dt_name: str, legalize: bool = True):
    repeat = int(os.environ.get("BASS_KERNEL_REPEAT", "1"))
    key = (dt_name, legalize, repeat)
    if key in _CACHED:
        return _CACHED[key]
    dt = {
        "bf16": mybir.dt.bfloat16,
        "f16": mybir.dt.float16,
        "f32": mybir.dt.float32,
    }[dt_name]

    nc = bass.Bass(
        "TRN2",
        target_bir_lowering=False,
        debug=False,
        num_devices=N_CORES,
    )
    x_ap = nc.dram_tensor("x", [128, XROWS, WP], dt, kind="ExternalInput").ap()
    kt_ap = nc.dram_tensor(
        "ktab", [128, KH * KW * DM], mybir.dt.float32, kind="ExternalInput"
    ).ap()
    o_aps = [
        nc.dram_tensor(f"o{di}", [128, HG, W], dt, kind="ExternalOutput").ap()
        for di in range(DM)
    ]

    from contextlib import ExitStack

    with tile.TileContext(nc) as tc, ExitStack() as ctx:
        kpool = ctx.enter_context(tc.tile_pool(name="kp", bufs=1))
        xpool = ctx.enter_context(tc.tile_pool(name="xp", bufs=1))
        tpool = ctx.enter_context(tc.tile_pool(name="tp", bufs=16))
        apool = ctx.enter_context(tc.tile_pool(name="acp", bufs=4))

        for _rep in range(repeat):
            # ktab rides the ACT HWDGE queue (tiny + every compute op reads
            # it) so block0's x DMA heads the SP queue.
            ktile = kpool.tile(
                [128, KH * KW * DM], mybir.dt.float32, name="ktile"
            )
            nc.scalar.dma_start(ktile[:], kt_ap[:])

            # One x tile holding all 66 input rows, DMAed in per-block
            # chunks so the first block's compute starts after ~1/4 of the
            # input lands. Tile sub-range deps gate each unit on the chunks
            # it actually reads (halo rows come from the neighbor chunk).
            # The first chunk is split across BOTH HWDGE queues (SP + ACT)
            # so it lands in half the time — it gates the whole ramp.
            xb = xpool.tile([128, XROWS, WP], dt, name="xb", tag="xb")
            h0 = (BLOCKS[0][1] + 2) // 2
            nc.sync.dma_start(xb[:, 0:h0], x_ap[:, 0:h0, :])
            nc.scalar.dma_start(
                xb[:, h0 : BLOCKS[0][1] + 2], x_ap[:, h0 : BLOCKS[0][1] + 2, :]
            )
            prev = BLOCKS[0][1] + 2
            for r0, rows in BLOCKS[1:]:
                hi = min(r0 + rows + 2, XROWS)
                nc.sync.dma_start(xb[:, prev:hi], x_ap[:, prev:hi, :])
                prev = hi

            # ACT warmup: zero-dependency ACTIVATE to hang the
            # activation-table load on, overlapping the input DMA.
            warm = kpool.tile([128, 1], dt, name="warm")
            nc.gpsimd.memset(warm[:], 0.0)
            nc.scalar.add(warm[:], warm[:], 0.0)

            def kvec(i, j, di):
                t = di * 9 + i * 3 + j
                return ktile[:, t : t + 1]

            n_bd = len(BLOCKS) * DM

            def phase_a(r0, rows, di, a):
                """All 9 tap adds (DVE TS + ACT) into acc + tmp tiles."""

                def src(i, j):
                    return xb[:, r0 + i : r0 + i + rows, j : j + W]

                n_dve_tmp = 8 - a
                act_taps = J1_TAPS + ALIGNED[: a - 3]
                dve_taps = ALIGNED[a - 3 : a - 3 + n_dve_tmp]

                acc = apool.tile([128, rows, W], dt, name="acc", tag="acc")
                i, j = INIT_TAP
                nc.vector.tensor_scalar_add(acc[:], src(i, j), kvec(i, j, di))
                tmps = []
                for i, j in dve_taps:
                    tmpv = tpool.tile(
                        [128, rows, W], dt, name="tmpv", tag="tmp"
                    )
                    nc.vector.tensor_scalar_add(
                        tmpv[:], src(i, j), kvec(i, j, di)
                    )
                    tmps.append(tmpv)
                for i, j in act_taps:
                    tmpa = tpool.tile(
                        [128, rows, W], dt, name="tmpa", tag="tmp"
                    )
                    nc.scalar.add(tmpa[:], src(i, j), kvec(i, j, di))
                    tmps.append(tmpa)
                return acc, tmps

            def phase_b(r0, rows, di, acc, merge, last_bd):
                """DVE merge of acc with remaining tmps + output DMA."""
                if not last_bd:
                    for t in merge:
                        nc.vector.tensor_max(acc[:], acc[:], t[:])
                    nc.sync.dma_start(o_aps[di][:, r0 : r0 + rows, :], acc[:])
                else:
                    # tail: run the whole merge chain + DMA in row slices
                    # so each slice's output DMA overlaps the next slice's
                    # merges and the kernel drains in sub-unit steps.
                    for s0 in range(0, rows, TAIL_SPLIT):
                        s1 = min(s0 + TAIL_SPLIT, rows)
                        for t in merge:
                            nc.vector.tensor_max(
                                acc[:, s0:s1], acc[:, s0:s1], t[:, s0:s1]
                            )
                        nc.sync.dma_start(
                            o_aps[di][:, r0 + s0 : r0 + s1, :], acc[:, s0:s1]
                        )

            # Software-pipelined emission: unit N's merges (phase B) are
            # issued AFTER unit N+1's adds (phase A), so each in-order
            # engine queue always has independent add work to run while a
            # unit's merge operands (ACT tmps, Pool partial) finish.
            units = [
                (blk, r0, rows, di)
                for blk, (r0, rows) in enumerate(BLOCKS)
                for di in range(DM)
            ]
            pending = None
            for bd, (_, r0, rows, di) in enumerate(units):
                a = UNIT_PLAN[bd]
                ab = phase_a(r0, rows, di, a)
                if pending is not None:
                    phase_b(*pending, last_bd=False)
                pending = (r0, rows, di, *ab)
            phase_b(*pending, last_bd=True)

    if legalize:
        _legalize_waits(nc)
    _CACHED[key] = nc
    return nc


def _host_prep(x: np.ndarray, kern: np.ndarray, dt_name: str):
    """Build per-core input maps."""
    np_dt = {"bf16": BF16, "f16": np.float16, "f32": np.float32}[dt_name]
    pad = PAD_VAL_F16 if dt_name == "f16" else PAD_VAL
    xp = np.full((B, H + 2, W + 2, C), pad, np.float32)
    xp[:, 1 : H + 1, 1 : W + 1, :] = x
    # (B, C, H+2, W+2), cast first to halve the transpose copy
    xp_t = np.ascontiguousarray(xp.astype(np_dt).transpose(0, 3, 1, 2))

    # ktab[g*64+c, di*9+i*3+j] = kern[i,j,c,di]
    ktap = np.transpose(kern.astype(np.float32), (2, 3, 0, 1)).reshape(C, DM * 9)
    ktab = np.ascontiguousarray(np.tile(ktap, (G, 1)))

    in_maps = []
    for core in range(N_CORES):
        b, g2 = core // 2, core % 2
        xa = np.empty((128, XROWS, WP), np_dt)
        for g in range(G):
            r0 = g2 * 128 + g * HG
            xa[g * C : (g + 1) * C] = xp_t[b, :, r0 : r0 + XROWS, :]
        in_maps.append({"x": xa, "ktab": ktab})
    return in_maps


def _assemble(results):
    out = np.empty((B, H, W, DM * C), np.float32)
    for core in range(N_CORES):
        b, g2 = core // 2, core % 2
        for di in range(DM):
            o = np.asarray(results[core][f"o{di}"]).astype(np.float32)
            o4 = o.reshape(G, C, HG, W)
            for g in range(G):
                h0 = g2 * 128 + g * HG
                out[b, h0 : h0 + HG, :, di * C : (di + 1) * C] = o4[g].transpose(
                    1, 2, 0
                )
    return out


def run(x, kern, trace=False):
    """Run on hardware; returns (output, BassKernelResults)."""
    nc = _build_bass(KERNEL_DT)
    in_maps = _host_prep(np.asarray(x, np.float32), np.asarray(kern, np.float32),
                         KERNEL_DT)
    r = run_bass_kernel_spmd(nc, in_maps, list(range(N_CORES)), trace=trace)
    return _assemble(r.results), r


def kernel(x, kernel):
    out, _ = run(x, kernel)
    return out



# revision 26
# speedup vs baseline: 1.0207x; 1.0011x over previous
"""Trainium2 Bass kernel for depthwise morphological dilation 2D (max-plus).

Problem (hardcoded):
  x:      (4, 256, 256, 64)  f32
  kernel: (3, 3, 64, 2)      f32
  out[b,y,x, di*64+c] = max_{i,j} x[b, y+i-1, x+j-1, c] + kernel[i,j,c,di]
  (SAME padding, stride 1), out: (4, 256, 256, 128) f32

Sharding: 8 cores, core k -> (batch b = k//2, H-half g2 = k%2), each core
computes 128 output rows (halo of 1 row each side handled host-side by
overlapped slicing of a padded array).

Device layout: SBUF partition p = g*64 + c  (g in {0,1} = row-subgroup of 64
output rows, c = channel). Free dims = (h, w) plane of one channel. The
per-channel kernel taps become per-partition [128,1] scalars.

Per tap (i,j,di): tmp = x_plane[i:i+R, j:j+256] + k[i,j,c,di]; acc = max(acc, tmp).
Adds run on ScalarE (activation Identity with per-partition bias; the j==1
taps go there since their odd-element offset breaks DVE's 4B-alignment fast
modes) and on DVE (tensor_scalar at 4x). Maxes run on DVE (tensor_tensor at
2x, 16-bit). 46 of the 72 adds on ACT balances both engines at ~167-170us
busy — the stock-op floor. Offload paths that do NOT work on the walrus
compile path (all verified): Pool/gpsimd elementwise (v3 ISA engine check
rejects TT/TensorReduce/Pool opcodes on Pool), DMA-CCE accum max (verifier
rejects), custom DVE fused add+max (cost model and uop tables cap at 1
elem/cyc — loses to the 0.25+0.5 split), PE ones-row adds (PSUM operand
drops the merge TT to 1x, negating the gain).

Schedule (sim 179.0us vs 182.7 prior / 190.6 original):
  - one xb tile, DMAed in per-block row chunks; block0's chunk is split
    across BOTH HWDGE queues (SP + ACT) so first compute starts ~2.3us in.
  - 16-row blocks -> 8 (block, di) units; two units' tmps fit SBUF, so
    emission is software-pipelined: unit N's merges are issued AFTER unit
    N+1's adds — each in-order engine queue always has independent add
    work while merge operands finish (kills the per-unit stalls).
  - last unit's merge chain + output DMA run in 8-row slices so the DMA
    drains overlapped with the final merges.
Correctness on HW: rel_err 3.4e-04.

Compute dtype fp16 (max-plus accumulates no rounding error — only
input/one-sum rounding, ~3e-4 rel err; fp16 pad is -60000, within range).
Outputs stored 16-bit and widened to f32 on host.
"""

import os
import sys

import numpy as np

# The Bass kernel executes via the axon/neuron PJRT backend; a CPU pin (used
# for running jax references) would hide the NeuronCores. Only effective if
# jax has not been imported yet in this process.
if "jax" not in sys.modules and os.environ.get("JAX_PLATFORMS") == "cpu":
    os.environ["JAX_PLATFORMS"] = ""

for _p in ("/opt/trn_rl_repo",):
    if _p not in sys.path:
        sys.path.insert(0, _p)

import ml_dtypes

from concourse import mybir
import concourse.bass as bass
import concourse.tile as tile
from concourse.bass_utils import run_bass_kernel_spmd

BF16 = ml_dtypes.bfloat16

B, H, W, C = 4, 256, 256, 64
KH, KW, DM = 3, 3, 2
N_CORES = 8
# -1e30 for f32/bf16; fp16 uses -60000 (finite, far below any real x+k)
PAD_VAL = np.float32(-1e30)
PAD_VAL_F16 = np.float32(-60000.0)

# device-side tiling
G = 2          # partition row-subgroups
HG = 64        # output rows per subgroup (per core: G*HG = 128)
WP = W + 2     # padded width
XROWS = HG + 2  # input rows per subgroup

KERNEL_DT = "f16"

# 16-row blocks: small enough that two units' worth of tmp tiles fit in
# SBUF (needed for the software-pipelined emission below), large enough
# to amortize ACT's 222-cycle per-op access overhead.
RB = 16
BLOCKS = [(r0, RB) for r0 in range(0, HG, RB)]
TAIL_SPLIT = 8  # last unit's merges+DMA run in 8-row slices

# Tap → engine assignment, per (block, di) unit.
#
# Work per unit: 9 tap adds (+k, per-partition scalar) and an 8-way max
# merge. Only DVE and ACT can do this work on the walrus/BIR compile
# path (Pool/gpsimd elementwise ops fail the v3 ISA engine check; DMA
# CCE rejects max; custom DVE ops are modeled/uncoded at 1x):
#   ACT — adds at 1 elem/cyc @1.2GHz; takes the j==1 taps (odd 2B
#         offset breaks DVE's 4B-alignment fast modes).
#   DVE — tensor_scalar adds at 4x, tensor_tensor maxes at 2x (fp16).
# LP balance: ~46 of the 72 adds on ACT, the rest + all 64 maxes on
# DVE → both engines ~167us busy.
INIT_TAP = (1, 0)
J1_TAPS = [(0, 1), (1, 1), (2, 1)]
ALIGNED = [(0, 0), (0, 2), (2, 0), (2, 2), (1, 2)]  # minus INIT_TAP
# ACT adds per unit (blk-major, di-minor); total 47 (ACT carries a
# little extra — it has tail slack while DVE is the binding engine).
# The last unit runs lighter ACT so its queue drains before the tail.
UNIT_PLAN = [6, 6, 6, 6, 6, 6, 5, 5]

_CACHED = {}


def _legalize_waits(nc):
    """Split multi-wait instructions: the TPB ISA allows one sem-wait per
    regular instruction (EventSemaphore holds 2). Tile's scheduler attaches
    all required waits to the consuming instruction; hoist the extras onto
    EventSemaphore instructions inserted just before it on the same engine
    queue (the queue is in-order, so semantics are unchanged)."""
    n_extra = 0
    for fn in nc.m.functions:
        for blk in fn.blocks:
            insts = blk.instructions
            new_list = []
            for ins in insts:
                si = ins.sync_info
                waits = list(si.on_wait) if (si and si.on_wait) else []
                if len(waits) > 1 and ins.opcode != "EventSemaphore":
                    keep, extra = waits[-1], waits[:-1]
                    for k in range(0, len(extra), 2):
                        es = mybir.InstEventSemaphore(
                            name=f"legalw_{ins.name}_{k}", ins=[], outs=[]
                        )
                        es.engine = ins.engine
                        es.sync_info = mybir.SyncInfo(
                            on_wait=extra[k : k + 2], on_update=[]
                        )
                        new_list.append(es)
                        n_extra += 1
                    si.on_wait = [keep]
                new_list.append(ins)
            insts[:] = new_list
    return n_extra


def _build_bass(dt_name: str, legalize: bool = True):
    repeat = int(os.environ.get("BASS_KERNEL_REPEAT", "1"))
    key = (dt_name, legalize, repeat)
    if key in _CACHED:
        return _CACHED[key]
    dt = {
        "bf16": mybir.dt.bfloat16,
        "f16": mybir.dt.float16,
        "f32": mybir.dt.float32,
    }[dt_name]

    nc = bass.Bass(
        "TRN2",
        target_bir_lowering=False,
        debug=False,
        num_devices=N_CORES,
    )
    x_ap = nc.dram_tensor("x", [128, XROWS, WP], dt, kind="ExternalInput").ap()
    kt_ap = nc.dram_tensor(
        "ktab", [128, KH * KW * DM], mybir.dt.float32, kind="ExternalInput"
    ).ap()
    o_aps = [
        nc.dram_tensor(f"o{di}", [128, HG, W], dt, kind="ExternalOutput").ap()
        for di in range(DM)
    ]

    from contextlib import ExitStack

    with tile.TileContext(nc) as tc, ExitStack() as ctx:
        kpool = ctx.enter_context(tc.tile_pool(name="kp", bufs=1))
        xpool = ctx.enter_context(tc.tile_pool(name="xp", bufs=1))
        tpool = ctx.enter_context(tc.tile_pool(name="tp", bufs=16))
        apool = ctx.enter_context(tc.tile_pool(name="acp", bufs=4))

        for _rep in range(repeat):
            # ktab rides the ACT HWDGE queue (tiny + every compute op reads
            # it) so block0's x DMA heads the SP queue.
            ktile = kpool.tile(
                [128, KH * KW * DM], mybir.dt.float32, name="ktile"
            )
            nc.scalar.dma_start(ktile[:], kt_ap[:])

            # One x tile holding all 66 input rows, DMAed in per-block
            # chunks so the first block's compute starts after ~1/4 of the
            # input lands. Tile sub-range deps gate each unit on the chunks
            # it actually reads (halo rows come from the neighbor chunk).
            # The first chunk is split across BOTH HWDGE queues (SP + ACT)
            # so it lands in half the time — it gates the whole ramp.
            xb = xpool.tile([128, XROWS, WP], dt, name="xb", tag="xb")
            h0 = (BLOCKS[0][1] + 2) // 2 + 1
            nc.sync.dma_start(xb[:, 0:h0], x_ap[:, 0:h0, :])
            nc.scalar.dma_start(
                xb[:, h0 : BLOCKS[0][1] + 2], x_ap[:, h0 : BLOCKS[0][1] + 2, :]
            )
            prev = BLOCKS[0][1] + 2
            for r0, rows in BLOCKS[1:]:
                hi = min(r0 + rows + 2, XROWS)
                nc.sync.dma_start(xb[:, prev:hi], x_ap[:, prev:hi, :])
                prev = hi

            # ACT warmup: zero-dependency ACTIVATE to hang the
            # activation-table load on, overlapping the input DMA.
            warm = kpool.tile([128, 1], dt, name="warm")
            nc.gpsimd.memset(warm[:], 0.0)
            nc.scalar.add(warm[:], warm[:], 0.0)

            def kvec(i, j, di):
                t = di * 9 + i * 3 + j
                return ktile[:, t : t + 1]

            n_bd = len(BLOCKS) * DM

            def phase_a(r0, rows, di, a):
                """All 9 tap adds (DVE TS + ACT) into acc + tmp tiles."""

                def src(i, j):
                    return xb[:, r0 + i : r0 + i + rows, j : j + W]

                n_dve_tmp = 8 - a
                act_taps = J1_TAPS + ALIGNED[: a - 3]
                dve_taps = ALIGNED[a - 3 : a - 3 + n_dve_tmp]

                acc = apool.tile([128, rows, W], dt, name="acc", tag="acc")
                i, j = INIT_TAP
                nc.vector.tensor_scalar_add(acc[:], src(i, j), kvec(i, j, di))
                tmps = []
                for i, j in dve_taps:
                    tmpv = tpool.tile(
                        [128, rows, W], dt, name="tmpv", tag="tmp"
                    )
                    nc.vector.tensor_scalar_add(
                        tmpv[:], src(i, j), kvec(i, j, di)
                    )
                    tmps.append(tmpv)
                for i, j in act_taps:
                    tmpa = tpool.tile(
                        [128, rows, W], dt, name="tmpa", tag="tmp"
                    )
                    nc.scalar.add(tmpa[:], src(i, j), kvec(i, j, di))
                    tmps.append(tmpa)
                return acc, tmps

            def phase_b(r0, rows, di, acc, merge, last_bd):
                """DVE merge of acc with remaining tmps + output DMA."""
                if not last_bd:
                    for t in merge:
                        nc.vector.tensor_max(acc[:], acc[:], t[:])
                    nc.sync.dma_start(o_aps[di][:, r0 : r0 + rows, :], acc[:])
                else:
                    # tail: run the whole merge chain + DMA in row slices
                    # so each slice's output DMA overlaps the next slice's
                    # merges and the kernel drains in sub-unit steps.
                    for s0 in range(0, rows, TAIL_SPLIT):
                        s1 = min(s0 + TAIL_SPLIT, rows)
                        for t in merge:
                            nc.vector.tensor_max(
                                acc[:, s0:s1], acc[:, s0:s1], t[:, s0:s1]
                            )
                        nc.sync.dma_start(
                            o_aps[di][:, r0 + s0 : r0 + s1, :], acc[:, s0:s1]
                        )

            # Software-pipelined emission: unit N's merges (phase B) are
            # issued AFTER unit N+1's adds (phase A), so each in-order
            # engine queue always has independent add work to run while a
            # unit's merge operands (ACT tmps, Pool partial) finish.
            units = [
                (blk, r0, rows, di)
                for blk, (r0, rows) in enumerate(BLOCKS)
                for di in range(DM)
            ]
            pending = None
            for bd, (_, r0, rows, di) in enumerate(units):
                a = UNIT_PLAN[bd]
                ab = phase_a(r0, rows, di, a)
                if pending is not None:
                    phase_b(*pending, last_bd=False)
                pending = (r0, rows, di, *ab)
            phase_b(*pending, last_bd=True)

    if legalize:
        _legalize_waits(nc)
    _CACHED[key] = nc
    return nc


def _host_prep(x: np.ndarray, kern: np.ndarray, dt_name: str):
    """Build per-core input maps."""
    np_dt = {"bf16": BF16, "f16": np.float16, "f32": np.float32}[dt_name]
    pad = PAD_VAL_F16 if dt_name == "f16" else PAD_VAL
    xp = np.full((B, H + 2, W + 2, C), pad, np.float32)
    xp[:, 1 : H + 1, 1 : W + 1, :] = x
    # (B, C, H+2, W+2), cast first to halve the transpose copy
    xp_t = np.ascontiguousarray(xp.astype(np_dt).transpose(0, 3, 1, 2))

    # ktab[g*64+c, di*9+i*3+j] = kern[i,j,c,di]
    ktap = np.transpose(kern.astype(np.float32), (2, 3, 0, 1)).reshape(C, DM * 9)
    ktab = np.ascontiguousarray(np.tile(ktap, (G, 1)))

    in_maps = []
    for core in range(N_CORES):
        b, g2 = core // 2, core % 2
        xa = np.empty((128, XROWS, WP), np_dt)
        for g in range(G):
            r0 = g2 * 128 + g * HG
            xa[g * C : (g + 1) * C] = xp_t[b, :, r0 : r0 + XROWS, :]
        in_maps.append({"x": xa, "ktab": ktab})
    return in_maps


def _assemble(results):
    out = np.empty((B, H, W, DM * C), np.float32)
    for core in range(N_CORES):
        b, g2 = core // 2, core % 2
        for di in range(DM):
            o = np.asarray(results[core][f"o{di}"]).astype(np.float32)
            o4 = o.reshape(G, C, HG, W)
            for g in range(G):
                h0 = g2 * 128 + g * HG
                out[b, h0 : h0 + HG, :, di * C : (di + 1) * C] = o4[g].transpose(
                    1, 2, 0
                )
    return out


def run(x, kern, trace=False):
    """Run on hardware; returns (output, BassKernelResults)."""
    nc = _build_bass(KERNEL_DT)
    in_maps = _host_prep(np.asarray(x, np.float32), np.asarray(kern, np.float32),
                         KERNEL_DT)
    r = run_bass_kernel_spmd(nc, in_maps, list(range(N_CORES)), trace=trace)
    return _assemble(r.results), r


def kernel(x, kernel):
    out, _ = run(x, kernel)
    return out



# revision 43
# speedup vs baseline: 1.0313x; 1.0104x over previous
"""Trainium2 Bass kernel for depthwise morphological dilation 2D (max-plus).

Problem (hardcoded):
  x:      (4, 256, 256, 64)  f32
  kernel: (3, 3, 64, 2)      f32
  out[b,y,x, di*64+c] = max_{i,j} x[b, y+i-1, x+j-1, c] + kernel[i,j,c,di]
  (SAME padding, stride 1), out: (4, 256, 256, 128) f32

Sharding: 8 cores, core k -> (batch b = k//2, H-half g2 = k%2), each core
computes 128 output rows (halo of 1 row each side handled host-side by
overlapped slicing of a padded array).

Device layout: SBUF partition p = g*64 + c  (g in {0,1} = row-subgroup of 64
output rows, c = channel). Free dims = (h, w) plane of one channel. The
per-channel kernel taps become per-partition [128,1] scalars.

Per tap (i,j,di): tmp = x_plane[i:i+R, j:j+256] + k[i,j,c,di]; acc = max(acc, tmp).
Adds run on ScalarE (activation Identity with per-partition bias; the j==1
taps go there since their odd-element offset breaks DVE's 4B-alignment fast
modes) and on DVE (tensor_scalar at 4x). Maxes run on DVE (tensor_tensor at
2x, 16-bit). 46 of the 72 adds on ACT balances both engines at ~167-170us
busy — the stock-op floor. Offload paths that do NOT work on the walrus
compile path (all verified): Pool/gpsimd elementwise (v3 ISA engine check
rejects TT/TensorReduce/Pool opcodes on Pool), DMA-CCE accum max (verifier
rejects), custom DVE fused add+max (cost model and uop tables cap at 1
elem/cyc — loses to the 0.25+0.5 split), PE ones-row adds (PSUM operand
drops the merge TT to 1x, negating the gain).

Schedule (sim 177.3us vs 182.7 prior baseline; ACT 95.3% / DVE 95.9% busy):
  - one xb tile, DMAed in per-block row chunks; block0's chunk is split
    across BOTH HWDGE queues (SP + ACT) so first compute starts ~2.3us in.
  - 16-row blocks -> 8 (block, di) units; two units' tmps fit SBUF, so
    emission is software-pipelined: unit N's merges are issued AFTER unit
    N+1's adds — each in-order engine queue always has independent add
    work while merge operands finish (kills the per-unit stalls).
  - first/last units run one fewer ACT add (ramp and drain are gated by
    ACT's serial queue there).
  - last unit: first 5 merges full-width, last 3 merges + output DMA in
    shrinking row slices [8,4,4] so each slice's DMA overlaps the next
    slice's merges and the final DMA is short.
Correctness on HW: rel_err 3.4e-04.

Compute dtype fp16 (max-plus accumulates no rounding error — only
input/one-sum rounding, ~3e-4 rel err; fp16 pad is -60000, within range).
Outputs stored 16-bit and widened to f32 on host.
"""

import os
import sys

import numpy as np

# The Bass kernel executes via the axon/neuron PJRT backend; a CPU pin (used
# for running jax references) would hide the NeuronCores. Only effective if
# jax has not been imported yet in this process.
if "jax" not in sys.modules and os.environ.get("JAX_PLATFORMS") == "cpu":
    os.environ["JAX_PLATFORMS"] = ""

for _p in ("/opt/trn_rl_repo",):
    if _p not in sys.path:
        sys.path.insert(0, _p)

import ml_dtypes

from concourse import mybir
import concourse.bass as bass
import concourse.tile as tile
from concourse.bass_utils import run_bass_kernel_spmd

BF16 = ml_dtypes.bfloat16

B, H, W, C = 4, 256, 256, 64
KH, KW, DM = 3, 3, 2
N_CORES = 8
# -1e30 for f32/bf16; fp16 uses -60000 (finite, far below any real x+k)
PAD_VAL = np.float32(-1e30)
PAD_VAL_F16 = np.float32(-60000.0)

# device-side tiling
G = 2          # partition row-subgroups
HG = 64        # output rows per subgroup (per core: G*HG = 128)
WP = W + 2     # padded width
XROWS = HG + 2  # input rows per subgroup

KERNEL_DT = "f16"

# 16-row blocks: small enough that two units' worth of tmp tiles fit in
# SBUF (needed for the software-pipelined emission below), large enough
# to amortize ACT's 222-cycle per-op access overhead.
RB = 16
BLOCKS = [(r0, RB) for r0 in range(0, HG, RB)]
TAIL_SLICES = [8, 4, 4]  # last unit's sliced-merge row steps
TAIL_MERGES = 3   # how many final merges run per-slice

# Tap → engine assignment, per (block, di) unit.
#
# Work per unit: 9 tap adds (+k, per-partition scalar) and an 8-way max
# merge. Only DVE and ACT can do this work on the walrus/BIR compile
# path (Pool/gpsimd elementwise ops fail the v3 ISA engine check; DMA
# CCE rejects max; custom DVE ops are modeled/uncoded at 1x):
#   ACT — adds at 1 elem/cyc @1.2GHz; takes the j==1 taps (odd 2B
#         offset breaks DVE's 4B-alignment fast modes).
#   DVE — tensor_scalar adds at 4x, tensor_tensor maxes at 2x (fp16).
# LP balance: ~46 of the 72 adds on ACT, the rest + all 64 maxes on
# DVE → both engines ~167us busy.
INIT_TAP = (1, 0)
J1_TAPS = [(0, 1), (1, 1), (2, 1)]
ALIGNED = [(0, 0), (0, 2), (2, 0), (2, 2), (1, 2)]  # minus INIT_TAP
# ACT adds per unit (blk-major, di-minor); total 46. First unit runs
# lighter ACT (its adds gate the pipeline ramp); last unit runs lighter
# ACT so its queue drains before the kernel tail.
UNIT_PLAN = [5, 6, 6, 6, 6, 6, 6, 5]

_CACHED = {}


def _legalize_waits(nc):
    """Split multi-wait instructions: the TPB ISA allows one sem-wait per
    regular instruction (EventSemaphore holds 2). Tile's scheduler attaches
    all required waits to the consuming instruction; hoist the extras onto
    EventSemaphore instructions inserted just before it on the same engine
    queue (the queue is in-order, so semantics are unchanged)."""
    n_extra = 0
    for fn in nc.m.functions:
        for blk in fn.blocks:
            insts = blk.instructions
            new_list = []
            for ins in insts:
                si = ins.sync_info
                waits = list(si.on_wait) if (si and si.on_wait) else []
                if len(waits) > 1 and ins.opcode != "EventSemaphore":
                    keep, extra = waits[-1], waits[:-1]
                    for k in range(0, len(extra), 2):
                        es = mybir.InstEventSemaphore(
                            name=f"legalw_{ins.name}_{k}", ins=[], outs=[]
                        )
                        es.engine = ins.engine
                        es.sync_info = mybir.SyncInfo(
                            on_wait=extra[k : k + 2], on_update=[]
                        )
                        new_list.append(es)
                        n_extra += 1
                    si.on_wait = [keep]
                new_list.append(ins)
            insts[:] = new_list
    return n_extra


def _build_bass(dt_name: str, legalize: bool = True):
    repeat = int(os.environ.get("BASS_KERNEL_REPEAT", "1"))
    key = (dt_name, legalize, repeat)
    if key in _CACHED:
        return _CACHED[key]
    dt = {
        "bf16": mybir.dt.bfloat16,
        "f16": mybir.dt.float16,
        "f32": mybir.dt.float32,
    }[dt_name]

    nc = bass.Bass(
        "TRN2",
        target_bir_lowering=False,
        debug=False,
        num_devices=N_CORES,
    )
    # Row 0 of x carries the 18 per-partition kernel taps (as fp16; k in
    # [-1,0] so the extra rounding is ~2e-4 abs, well inside the error
    # budget). This removes the separate ktab DMA — each DMACopy carries a
    # ~1.7us fixed cost in the model and the ktab one sat at the head of
    # the ACT HWDGE queue, serially delaying the block0 input chunk.
    x_ap = nc.dram_tensor(
        "x", [128, XROWS + 1, WP], dt, kind="ExternalInput"
    ).ap()
    o_aps = [
        nc.dram_tensor(f"o{di}", [128, HG, W], dt, kind="ExternalOutput").ap()
        for di in range(DM)
    ]

    from contextlib import ExitStack

    with tile.TileContext(nc) as tc, ExitStack() as ctx:
        kpool = ctx.enter_context(tc.tile_pool(name="kp", bufs=1))
        xpool = ctx.enter_context(tc.tile_pool(name="xp", bufs=1))
        tpool = ctx.enter_context(tc.tile_pool(name="tp", bufs=16))
        apool = ctx.enter_context(tc.tile_pool(name="acp", bufs=4))

        for _rep in range(repeat):
            # One x tile holding the ktab row + all 66 input rows, DMAed in
            # per-block chunks so the first block's compute starts after
            # ~1/4 of the input lands. Tile sub-range deps gate each unit
            # on the chunks it actually reads (halo rows come from the
            # neighbor chunk). The first chunk is split across BOTH HWDGE
            # queues (SP + ACT) so it lands in half the time — it gates
            # the whole ramp. (Input row r lives at tile row r+1.)
            xb = xpool.tile([128, XROWS + 1, WP], dt, name="xb", tag="xb")
            h0 = (BLOCKS[0][1] + 3) // 2 + 1
            nc.sync.dma_start(xb[:, 0:h0], x_ap[:, 0:h0, :])
            nc.scalar.dma_start(
                xb[:, h0 : BLOCKS[0][1] + 3], x_ap[:, h0 : BLOCKS[0][1] + 3, :]
            )
            prev = BLOCKS[0][1] + 3
            for r0, rows in BLOCKS[1:]:
                hi = min(r0 + rows + 3, XROWS + 1)
                nc.sync.dma_start(xb[:, prev:hi], x_ap[:, prev:hi, :])
                prev = hi

            # ACT warmup: zero-dependency ACTIVATE to hang the
            # activation-table load on, overlapping the input DMA.
            warm = kpool.tile([128, 1], dt, name="warm")
            nc.gpsimd.memset(warm[:], 0.0)
            nc.scalar.add(warm[:], warm[:], 0.0)

            def kvec(i, j, di):
                # row 0 holds the f32 ktab bitcast into fp16-pair slots
                t = di * 9 + i * 3 + j
                return xb[:, 0, 2 * t : 2 * t + 2].bitcast(mybir.dt.float32)

            n_bd = len(BLOCKS) * DM

            def phase_a(r0, rows, di, a):
                """All 9 tap adds (DVE TS + ACT) into acc + tmp tiles."""

                def src(i, j):
                    return xb[:, 1 + r0 + i : 1 + r0 + i + rows, j : j + W]

                n_dve_tmp = 8 - a
                act_taps = J1_TAPS + ALIGNED[: a - 3]
                dve_taps = ALIGNED[a - 3 : a - 3 + n_dve_tmp]

                acc = apool.tile([128, rows, W], dt, name="acc", tag="acc")
                i, j = INIT_TAP
                nc.vector.tensor_scalar_add(acc[:], src(i, j), kvec(i, j, di))
                tmps = []
                for i, j in dve_taps:
                    tmpv = tpool.tile(
                        [128, rows, W], dt, name="tmpv", tag="tmp"
                    )
                    nc.vector.tensor_scalar_add(
                        tmpv[:], src(i, j), kvec(i, j, di)
                    )
                    tmps.append(tmpv)
                for i, j in act_taps:
                    tmpa = tpool.tile(
                        [128, rows, W], dt, name="tmpa", tag="tmp"
                    )
                    nc.scalar.add(tmpa[:], src(i, j), kvec(i, j, di))
                    tmps.append(tmpa)
                return acc, tmps

            def phase_b(r0, rows, di, acc, merge, last_bd):
                """DVE merge of acc with remaining tmps + output DMA."""
                if not last_bd:
                    for t in merge:
                        nc.vector.tensor_max(acc[:], acc[:], t[:])
                    nc.sync.dma_start(o_aps[di][:, r0 : r0 + rows, :], acc[:])
                else:
                    # tail: first merges run full-width (amortizes the
                    # 58-cycle DVE access cost); only the last TAIL_MERGES
                    # merges + the output DMA run in shrinking row slices,
                    # so each slice's DMA overlaps the next slice's merges.
                    for t in merge[:-TAIL_MERGES]:
                        nc.vector.tensor_max(acc[:], acc[:], t[:])
                    s0 = 0
                    for sz in TAIL_SLICES:
                        s1 = min(s0 + sz, rows)
                        for t in merge[-TAIL_MERGES:]:
                            nc.vector.tensor_max(
                                acc[:, s0:s1], acc[:, s0:s1], t[:, s0:s1]
                            )
                        nc.sync.dma_start(
                            o_aps[di][:, r0 + s0 : r0 + s1, :], acc[:, s0:s1]
                        )
                        s0 = s1
                        if s0 >= rows:
                            break

            # Software-pipelined emission: unit N's merges (phase B) are
            # issued AFTER unit N+1's adds (phase A), so each in-order
            # engine queue always has independent add work to run while a
            # unit's merge operands (ACT tmps, Pool partial) finish.
            units = [
                (blk, r0, rows, di)
                for blk, (r0, rows) in enumerate(BLOCKS)
                for di in range(DM)
            ]
            pending = None
            for bd, (_, r0, rows, di) in enumerate(units):
                a = UNIT_PLAN[bd]
                ab = phase_a(r0, rows, di, a)
                if pending is not None:
                    phase_b(*pending, last_bd=False)
                pending = (r0, rows, di, *ab)
            phase_b(*pending, last_bd=True)

    if legalize:
        _legalize_waits(nc)
    _CACHED[key] = nc
    return nc


def _host_prep(x: np.ndarray, kern: np.ndarray, dt_name: str):
    """Build per-core input maps."""
    np_dt = {"bf16": BF16, "f16": np.float16, "f32": np.float32}[dt_name]
    pad = PAD_VAL_F16 if dt_name == "f16" else PAD_VAL
    xp = np.full((B, H + 2, W + 2, C), pad, np.float32)
    xp[:, 1 : H + 1, 1 : W + 1, :] = x
    # (B, C, H+2, W+2), cast first to halve the transpose copy
    xp_t = np.ascontiguousarray(xp.astype(np_dt).transpose(0, 3, 1, 2))

    # ktab[g*64+c, di*9+i*3+j] = kern[i,j,c,di]; rides in tile row 0 of x
    # as raw f32 bytes viewed as fp16 pairs (the engines' scalar operands
    # must be f32 — the kernel reads them back via a bitcast AP).
    ktap = np.transpose(kern.astype(np.float32), (2, 3, 0, 1)).reshape(C, DM * 9)
    ktab = np.tile(ktap, (G, 1)).astype(np.float32).view(np.float16)

    in_maps = []
    for core in range(N_CORES):
        b, g2 = core // 2, core % 2
        xa = np.zeros((128, XROWS + 1, WP), np_dt)
        xa[:, 0, : DM * 9 * 2] = ktab
        for g in range(G):
            r0 = g2 * 128 + g * HG
            xa[g * C : (g + 1) * C, 1:] = xp_t[b, :, r0 : r0 + XROWS, :]
        in_maps.append({"x": xa})
    return in_maps


def _assemble(results):
    out = np.empty((B, H, W, DM * C), np.float32)
    for core in range(N_CORES):
        b, g2 = core // 2, core % 2
        for di in range(DM):
            o = np.asarray(results[core][f"o{di}"]).astype(np.float32)
            o4 = o.reshape(G, C, HG, W)
            for g in range(G):
                h0 = g2 * 128 + g * HG
                out[b, h0 : h0 + HG, :, di * C : (di + 1) * C] = o4[g].transpose(
                    1, 2, 0
                )
    return out


def run(x, kern, trace=False):
    """Run on hardware; returns (output, BassKernelResults)."""
    nc = _build_bass(KERNEL_DT)
    in_maps = _host_prep(np.asarray(x, np.float32), np.asarray(kern, np.float32),
                         KERNEL_DT)
    r = run_bass_kernel_spmd(nc, in_maps, list(range(N_CORES)), trace=trace)
    return _assemble(r.results), r


def kernel(x, kernel):
    out, _ = run(x, kernel)
    return out

